# revision 15
# baseline (speedup 1.0000x reference)
"""Bahdanau additive-attention kernel for Trainium2, SPMD across 8 NeuronCores.

Reference computation (all fp32):
    q_proj  = query @ W1_w.T + W1_b            # [D]
    v_proj  = values @ W2_w.T + W2_b           # [T, D]
    weights = softmax(tanh(q_proj + v_proj) * v, axis=0)   # over T
    out     = weights * values                 # [T, D]

Sharding: values is split along T across 8 cores (2048 rows each); W2 is
replicated (shipped pre-transposed + pre-blocked in bf16); the q-projection
matvec is sharded over the contraction dim (each core handles 256 columns of
W1) and finished with an AllReduce; the softmax denominator (per-column sum
of exps) is AllReduced.  Logits are bounded in [-0.1, 0.1] (tanh * v with
|v| <= 0.1) so the softmax needs no max-subtraction pass.

Per-core device program:
  - VT (values shard transposed, bf16) resident in SBUF: the moving matmul
    operand; psum tiles are [d=128 part, t=512 free].
  - ScalarE: tanh(psum + qb[d]) then exp(v[d] * x) with accum_out giving the
    per-partition running sum of exps (softmax denominator) for free.
  - e stored fp16 in SBUF; transposed back to [t, d] via TensorE without
    waiting for the sum-exp AllReduce; 1/S is broadcast across partitions
    with a ones-matmul and folded into the final values multiply on DVE.
"""

import numpy as np

import concourse.bacc as bacc
import concourse.bass as bass
import concourse.tile as tile
from concourse import mybir
from concourse import masks
from concourse.bass_utils import run_bass_kernel_spmd

F32 = mybir.dt.float32
BF16 = mybir.dt.bfloat16
FP16 = mybir.dt.float16

D = 2048          # feature dim
T = 16384         # total timesteps
N_CORES = 8
TS = T // N_CORES  # timesteps per core = 2048
KS = D // N_CORES  # W1 contraction slice per core = 256


def build_kernel(D=D, TS=TS, KS=KS, n_cores=N_CORES, debug=False):
    DT = D // 128     # d-tiles of 128
    KT = D // 128     # k-tiles of 128
    TC = TS // 512    # t-chunks of 512
    IT = TS // 128    # t-tiles of 128
    DCW = min(512, D)  # d-chunk width in pass 2 (one PSUM bank)
    DC = D // DCW
    JJ = DCW // 128
    N_CORES_ = n_cores

    nc = bacc.Bacc(None, target_bir_lowering=False, debug=debug, num_devices=N_CORES_)

    # Per-core inputs (see make_in_maps for host-side layouts)
    vals = nc.dram_tensor("vals", [TS, D], F32, kind="ExternalInput")
    valsT = nc.dram_tensor("valsT", [D, TS], BF16, kind="ExternalInput")
    w2t = nc.dram_tensor("w2t", [DT, 128, KT * 128], BF16, kind="ExternalInput")
    w1c = nc.dram_tensor("w1c", [128, DT * KS], F32, kind="ExternalInput")
    qs = nc.dram_tensor("qs", [KS], F32, kind="ExternalInput")
    w1b = nc.dram_tensor("w1b", [D], F32, kind="ExternalInput")
    w2b = nc.dram_tensor("w2b", [D], F32, kind="ExternalInput")
    vvec = nc.dram_tensor("vvec", [D], F32, kind="ExternalInput")
    out = nc.dram_tensor("out", [TS, D], F32, kind="ExternalOutput")

    with tile.TileContext(nc) as tc:
        with (
            tc.tile_pool(name="const", bufs=1) as const_pool,
            tc.tile_pool(name="vt", bufs=1) as vt_pool,
            tc.tile_pool(name="e", bufs=1) as e_pool,
            tc.tile_pool(name="w2tb", bufs=2) as w2tb_pool,
            tc.tile_pool(name="st", bufs=2) as st_pool,
            tc.tile_pool(name="vn", bufs=2) as vn_pool,
            tc.tile_pool(name="osb", bufs=2) as osb_pool,
            tc.tile_pool(name="psum", bufs=6, space="PSUM") as psum_pool,
            tc.tile_pool(name="psumT", bufs=2, space="PSUM") as psumT_pool,
            tc.tile_pool(name="dram", bufs=1, space="DRAM") as dram_pool,
        ):
            # ---------------- constants / small vectors ----------------
            qbv = const_pool.tile([128, DT], F32)    # qb[d] laid out [p, dj]
            vv = const_pool.tile([128, DT], F32)     # v[d]
            Sloc = const_pool.tile([128, DT], F32)   # local sum-exp
            b1v = const_pool.tile([128, DT], F32)
            b2v = const_pool.tile([128, DT], F32)
            acc = const_pool.tile([128, DT * TC], F32)  # per (dj, tc) exp-sums
            ident = const_pool.tile([128, 128], FP16)
            ones1 = const_pool.tile([1, 128], F32)
            qs1 = const_pool.tile([1, KS], F32)
            qpart = const_pool.tile([128, DT], F32)  # local q_proj partial
            srow = const_pool.tile([1, D], F32)      # global sum-exp -> 1/S (in place)
            rbc = const_pool.tile([128, D], F32)     # 1/S broadcast across partitions

            masks.make_identity(nc, ident[:, :])
            nc.vector.memset(ones1[:, :], 1.0)

            # dram bounce buffers for the two collectives
            qb_in = dram_pool.tile([128, DT], F32)
            qb_out = dram_pool.tile([128, DT], F32)
            s_in = dram_pool.tile([D], F32)
            s_out = dram_pool.tile([D], F32)

            # ---------------- q-projection matvec (sharded over k) -----
            # issued first so its DMAs + AllReduce land before PSUM
            # backpressure makes the first tanh (which needs qb) critical.
            W1CH = min(4, DT)              # w1 chunks
            JCH = DT // W1CH               # dj tiles per chunk
            with tc.tile_pool(name="w1pool", bufs=2) as w1_pool:
                qsb = w1_pool.tile([128, KS], F32, bufs=1)
                junk = w1_pool.tile([128, KS], F32, bufs=1)
                nc.sync.dma_start(qs1[:, :], qs[:].rearrange("(a k) -> a k", a=1))
                psq = psumT_pool.tile([128, KS], F32, tag="pT")
                nc.tensor.matmul(psq[:, :], ones1[:, :], qs1[:, :], start=True, stop=True)
                nc.scalar.copy(qsb[:, :], psq[:, :])
                for ch in range(W1CH):
                    w1t = w1_pool.tile([128, JCH * KS], F32, tag="w1t")
                    nc.sync.dma_start(
                        w1t[:, :], w1c[:, ch * JCH * KS:(ch + 1) * JCH * KS])
                    for jj in range(JCH):
                        j = ch * JCH + jj
                        nc.vector.tensor_mul(
                            junk[:, :], w1t[:, jj * KS:(jj + 1) * KS], qsb[:, :])
                        nc.vector.tensor_reduce(
                            qpart[:, j:j + 1], junk[:, :],
                            axis=mybir.AxisListType.X, op=mybir.AluOpType.add)

                nc.sync.dma_start(qb_in[:, :], qpart[:, :])
                nc.gpsimd.collective_compute(
                    "AllReduce", mybir.AluOpType.add,
                    replica_groups=[list(range(N_CORES_))],
                    ins=[qb_in.opt()], outs=[qb_out.opt()],
                )

            # biases / v in [p, dj] layout: elem (p, j) <- dram[128j + p]
            nc.sync.dma_start(b1v[:, :], w1b[:].rearrange("(j p) -> p j", p=128))
            nc.sync.dma_start(b2v[:, :], w2b[:].rearrange("(j p) -> p j", p=128))
            nc.sync.dma_start(vv[:, :], vvec[:].rearrange("(j p) -> p j", p=128))
            nc.vector.tensor_add(b1v[:, :], b1v[:, :], b2v[:, :])  # off critical path
            nc.sync.dma_start(qbv[:, :], qb_out[:, :])
            nc.vector.tensor_add(qbv[:, :], qbv[:, :], b1v[:, :])

            # ---------------- VT resident load (bf16) -------------------
            # vt[kt][p, t] = values_s[t, 128*kt + p]
            vt_tiles = []
            for kt in range(KT):
                vt = vt_pool.tile([128, TS], BF16, name=f"vt{kt}")
                nc.sync.dma_start(vt[:, :], valsT[kt * 128:(kt + 1) * 128, :])
                vt_tiles.append(vt)

            # ---------------- pass 1: matmul + tanh + exp ---------------
            e_tiles = []
            for dj in range(DT):
                e_tiles.append(e_pool.tile([128, TS], FP16, name=f"e{dj}"))

            for dj in range(DT):
                w2tb = w2tb_pool.tile([128, KT * 128], BF16)
                nc.sync.dma_start(w2tb[:, :], w2t[dj, :, :])
                for tc_i in range(TC):
                    ps = psum_pool.tile([128, 512], F32)
                    for kt in range(KT):
                        nc.tensor.matmul(
                            ps[:, :],
                            w2tb[:, kt * 128:(kt + 1) * 128],
                            vt_tiles[kt][:, tc_i * 512:(tc_i + 1) * 512],
                            start=(kt == 0),
                            stop=(kt == KT - 1),
                        )
                    st = st_pool.tile([128, 512], F32)
                    nc.scalar.activation(
                        st[:, :], ps[:, :],
                        mybir.ActivationFunctionType.Tanh,
                        bias=qbv[:, dj:dj + 1], scale=1.0,
                    )
                    nc.scalar.activation(
                        e_tiles[dj][:, tc_i * 512:(tc_i + 1) * 512], st[:, :],
                        mybir.ActivationFunctionType.Exp,
                        bias=0.0, scale=vv[:, dj:dj + 1],
                        accum_out=acc[:, dj * TC + tc_i:dj * TC + tc_i + 1],
                    )
                nc.vector.tensor_reduce(
                    Sloc[:, dj:dj + 1],
                    acc[:, dj * TC:(dj + 1) * TC],
                    axis=mybir.AxisListType.X,
                    op=mybir.AluOpType.add,
                )

            # ---------------- sum-exp AllReduce + 1/S broadcast ---------
            # s_in is written d-major so the result can be read back as a row.
            nc.sync.dma_start(s_in[:].rearrange("(j p) -> p j", p=128), Sloc[:, :])
            nc.gpsimd.collective_compute(
                "AllReduce", mybir.AluOpType.add,
                replica_groups=[list(range(N_CORES_))],
                ins=[s_in.opt()], outs=[s_out.opt()],
            )
            nc.sync.dma_start(srow[:, :], s_out[:].rearrange("(a k) -> a k", a=1))
            nc.vector.reciprocal(srow[:, :], srow[:, :])
            for dc in range(DC):
                pb = psumT_pool.tile([128, DCW], F32, name="pbcast", tag="pT")
                nc.tensor.matmul(
                    pb[:, :], ones1[:, :], srow[:, dc * DCW:(dc + 1) * DCW],
                    start=True, stop=True)
                nc.scalar.copy(rbc[:, dc * DCW:(dc + 1) * DCW], pb[:, :])

            # ---------------- pass 2: transpose + multiply --------------
            # transposes read UNSCALED e so they don't wait on the AllReduce
            HW_ = max(DCW, D // 2)
            NHALF = D // HW_
            for it in range(IT):
                for half in range(NHALF):
                    vn = vn_pool.tile([128, HW_], F32)
                    nc.sync.dma_start(
                        vn[:, :],
                        vals[it * 128:(it + 1) * 128, half * HW_:(half + 1) * HW_])
                    osb = osb_pool.tile([128, HW_], F32)
                    for dch in range(HW_ // DCW):
                        dc = half * (HW_ // DCW) + dch
                        pst = psumT_pool.tile([128, DCW], FP16, name="pst", tag="pT")
                        for jj in range(JJ):
                            dj = dc * JJ + jj
                            nc.tensor.transpose(
                                pst[:, jj * 128:(jj + 1) * 128],
                                e_tiles[dj][:, it * 128:(it + 1) * 128],
                                ident[:, :],
                            )
                        # osb = eT * values, then * (1/S)
                        nc.vector.tensor_mul(
                            osb[:, dch * DCW:(dch + 1) * DCW],
                            pst[:, :], vn[:, dch * DCW:(dch + 1) * DCW])
                        nc.vector.tensor_mul(
                            osb[:, dch * DCW:(dch + 1) * DCW],
                            osb[:, dch * DCW:(dch + 1) * DCW],
                            rbc[:, dc * DCW:(dc + 1) * DCW])
                    nc.sync.dma_start(
                        out[it * 128:(it + 1) * 128, half * HW_:(half + 1) * HW_],
                        osb[:, :])

    nc.compile()
    return nc


_NC_CACHE = None


def _get_nc():
    global _NC_CACHE
    if _NC_CACHE is None:
        _NC_CACHE = build_kernel()
    return _NC_CACHE


def make_in_maps(query, values, v, W1_w, W1_b, W2_w, W2_b,
                 D_=None, TS_=None, KS_=None, n_cores=N_CORES):
    import ml_dtypes
    D_ = D_ or D
    TS_ = TS_ or TS
    KS_ = KS_ or KS
    DT_ = D_ // 128
    KT_ = D_ // 128
    # w2t blocked: B[dj, p, kt, f] = W2_w[128dj+f, 128kt+p]
    w2t_blocked = np.ascontiguousarray(
        W2_w.reshape(DT_, 128, KT_, 128).transpose(0, 3, 2, 1)
        .reshape(DT_, 128, KT_ * 128).astype(ml_dtypes.bfloat16))
    in_maps = []
    for c in range(n_cores):
        vs = np.ascontiguousarray(values[c * TS_:(c + 1) * TS_])
        w1s = W1_w[:, c * KS_:(c + 1) * KS_]
        w1blocked = np.ascontiguousarray(
            w1s.reshape(DT_, 128, KS_).transpose(1, 0, 2).reshape(128, DT_ * KS_))
        in_maps.append({
            "vals": vs,
            "valsT": np.ascontiguousarray(vs.T.astype(ml_dtypes.bfloat16)),
            "w2t": w2t_blocked,
            "w1c": w1blocked,
            "qs": np.ascontiguousarray(query[c * KS_:(c + 1) * KS_]),
            "w1b": W1_b,
            "w2b": W2_b,
            "vvec": v,
        })
    return in_maps


def kernel(query, values, v, W1_w, W1_b, W2_w, W2_b, _trace=False, _trace_kwargs=None):
    query = np.asarray(query, np.float32)
    values = np.asarray(values, np.float32)
    v = np.asarray(v, np.float32)
    W1_w = np.asarray(W1_w, np.float32)
    W1_b = np.asarray(W1_b, np.float32)
    W2_w = np.asarray(W2_w, np.float32)
    W2_b = np.asarray(W2_b, np.float32)

    nc = _get_nc()
    in_maps = make_in_maps(query, values, v, W1_w, W1_b, W2_w, W2_b)
    res = run_bass_kernel_spmd(
        nc, in_maps, core_ids=list(range(N_CORES)),
        trace=_trace, **(_trace_kwargs or {}),
    )
    shards = [np.asarray(om["out"], np.float32) for om in res.results]
    out = np.concatenate(shards, axis=0)
    if _trace:
        return out, res
    return out


# revision 17
# speedup vs baseline: 1.0423x; 1.0423x over previous
"""Bahdanau additive-attention kernel for Trainium2, SPMD across 8 NeuronCores.

Reference computation (all fp32):
    q_proj  = query @ W1_w.T + W1_b            # [D]
    v_proj  = values @ W2_w.T + W2_b           # [T, D]
    weights = softmax(tanh(q_proj + v_proj) * v, axis=0)   # over T
    out     = weights * values                 # [T, D]

Sharding: values is split along T across 8 cores (2048 rows each); W2 is
replicated (shipped pre-transposed + pre-blocked in bf16); the q-projection
matvec is sharded over the contraction dim (each core handles 256 columns of
W1) and finished with an AllReduce; the softmax denominator (per-column sum
of exps) is AllReduced.  Logits are bounded in [-0.1, 0.1] (tanh * v with
|v| <= 0.1) so the softmax needs no max-subtraction pass.

Per-core device program:
  - VT (values shard transposed, bf16) resident in SBUF as the moving matmul
    operand; psum tiles are [d=128 part, t=512 free]; the k loop is OUTER so
    the first tiles stream at DMA pace and the stationary operand is reused
    across 4 consecutive matmuls.
  - ScalarE: tanh(psum + qb[d]) then exp(v[d] * x) with accum_out giving the
    per-partition running sum of exps (softmax denominator) for free.
  - e stored fp16 in SBUF.  Pass 2: e *= 1/S[d] (per-partition tensor_scalar,
    in place), outT = e * valuesT(fp32) on DVE, TensorE transposes outT back
    to [t, d], ScalarE evacuates PSUM to SBUF, DMA out.
"""

import numpy as np

import concourse.bacc as bacc
import concourse.bass as bass
import concourse.tile as tile
from concourse import mybir
from concourse import masks
from concourse.bass_utils import run_bass_kernel_spmd

F32 = mybir.dt.float32
BF16 = mybir.dt.bfloat16
FP16 = mybir.dt.float16

D = 2048          # feature dim
T = 16384         # total timesteps
N_CORES = 8
TS = T // N_CORES  # timesteps per core = 2048
KS = D // N_CORES  # W1 contraction slice per core = 256


def build_kernel(D=D, TS=TS, KS=KS, n_cores=N_CORES, debug=False):
    DT = D // 128     # d-tiles of 128
    KT = D // 128     # k-tiles of 128
    TC = TS // 512    # t-chunks of 512
    IT = TS // 128    # t-tiles of 128
    GJ = min(4, DT)   # dj per pass-2 group (one 512-wide d-chunk)
    NG = DT // GJ     # number of pass-2 groups
    THW = min(1024, TS)  # pass-2 t-half width
    NTH = TS // THW
    N_CORES_ = n_cores

    nc = bacc.Bacc(None, target_bir_lowering=False, debug=debug, num_devices=N_CORES_)

    # Per-core inputs (see make_in_maps for host-side layouts)
    valsTf = nc.dram_tensor("valsTf", [D, TS], F32, kind="ExternalInput")
    valsT = nc.dram_tensor("valsT", [D, TS], BF16, kind="ExternalInput")
    w2t = nc.dram_tensor("w2t", [DT, 128, KT * 128], BF16, kind="ExternalInput")
    w1c = nc.dram_tensor("w1c", [128, DT * KS], F32, kind="ExternalInput")
    qs = nc.dram_tensor("qs", [KS], F32, kind="ExternalInput")
    w1b = nc.dram_tensor("w1b", [D], F32, kind="ExternalInput")
    w2b = nc.dram_tensor("w2b", [D], F32, kind="ExternalInput")
    vvec = nc.dram_tensor("vvec", [D], F32, kind="ExternalInput")
    out = nc.dram_tensor("out", [TS, D], F32, kind="ExternalOutput")

    with tile.TileContext(nc) as tc:
        with (
            tc.tile_pool(name="const", bufs=1) as const_pool,
            tc.tile_pool(name="vt", bufs=1) as vt_pool,
            tc.tile_pool(name="e", bufs=1) as e_pool,
            tc.tile_pool(name="w2tb", bufs=2) as w2tb_pool,
            tc.tile_pool(name="st", bufs=2) as st_pool,
            tc.tile_pool(name="vtn", bufs=2) as vtn_pool,
            tc.tile_pool(name="outT", bufs=2 * GJ) as outT_pool,
            tc.tile_pool(name="osb", bufs=3) as osb_pool,
            tc.tile_pool(name="psum", bufs=6, space="PSUM") as psum_pool,
            tc.tile_pool(name="psumT", bufs=2, space="PSUM") as psumT_pool,
            tc.tile_pool(name="dram", bufs=1, space="DRAM") as dram_pool,
        ):
            # ---------------- constants / small vectors ----------------
            qbv = const_pool.tile([128, DT], F32)    # qb[d] laid out [p, dj]
            vv = const_pool.tile([128, DT], F32)     # v[d]
            rv = const_pool.tile([128, DT], F32)     # 1/S[d]
            Sloc = const_pool.tile([128, DT], F32)   # local sum-exp
            b1v = const_pool.tile([128, DT], F32)
            b2v = const_pool.tile([128, DT], F32)
            acc = const_pool.tile([128, DT * TC], F32)  # per (dj, tc) exp-sums
            ident = const_pool.tile([128, 128], F32)
            ones1 = const_pool.tile([1, 128], F32)
            qs1 = const_pool.tile([1, KS], F32)
            qpart = const_pool.tile([128, DT], F32)  # local q_proj partial

            masks.make_identity(nc, ident[:, :])
            nc.vector.memset(ones1[:, :], 1.0)

            # dram bounce buffers for the two collectives
            qb_in = dram_pool.tile([128, DT], F32)
            qb_out = dram_pool.tile([128, DT], F32)
            s_in = dram_pool.tile([128, DT], F32)
            s_out = dram_pool.tile([128, DT], F32)

            # ---------------- q-projection matvec (sharded over k) -----
            # Feeder DMAs go through gpsimd (SWDGE) so the AllReduce is not
            # queued behind the bulk HWDGE loads.
            W1CH = min(4, DT)
            JCH = DT // W1CH
            with tc.tile_pool(name="w1pool", bufs=2) as w1_pool:
                qsb = w1_pool.tile([128, KS], F32, bufs=1)
                junk = w1_pool.tile([128, KS], F32, bufs=1)
                nc.gpsimd.dma_start(qs1[:, :], qs[:].rearrange("(a k) -> a k", a=1))
                psq = psumT_pool.tile([128, KS], F32, tag="pT")
                nc.tensor.matmul(psq[:, :], ones1[:, :], qs1[:, :], start=True, stop=True)
                nc.scalar.copy(qsb[:, :], psq[:, :])
                for ch in range(W1CH):
                    w1t = w1_pool.tile([128, JCH * KS], F32, tag="w1t")
                    nc.gpsimd.dma_start(
                        w1t[:, :], w1c[:, ch * JCH * KS:(ch + 1) * JCH * KS])
                    for jj in range(JCH):
                        j = ch * JCH + jj
                        nc.vector.tensor_mul(
                            junk[:, :], w1t[:, jj * KS:(jj + 1) * KS], qsb[:, :])
                        nc.vector.tensor_reduce(
                            qpart[:, j:j + 1], junk[:, :],
                            axis=mybir.AxisListType.X, op=mybir.AluOpType.add)

                nc.gpsimd.dma_start(qb_in[:, :], qpart[:, :])
                nc.gpsimd.collective_compute(
                    "AllReduce", mybir.AluOpType.add,
                    replica_groups=[list(range(N_CORES_))],
                    ins=[qb_in.opt()], outs=[qb_out.opt()],
                )

            # biases / v in [p, dj] layout: elem (p, j) <- dram[128j + p]
            nc.gpsimd.dma_start(b1v[:, :], w1b[:].rearrange("(j p) -> p j", p=128))
            nc.gpsimd.dma_start(b2v[:, :], w2b[:].rearrange("(j p) -> p j", p=128))
            nc.gpsimd.dma_start(vv[:, :], vvec[:].rearrange("(j p) -> p j", p=128))
            nc.vector.tensor_add(b1v[:, :], b1v[:, :], b2v[:, :])  # off critical path
            nc.gpsimd.dma_start(qbv[:, :], qb_out[:, :])
            nc.vector.tensor_add(qbv[:, :], qbv[:, :], b1v[:, :])

            # ---------------- VT resident load (bf16) -------------------
            # vt[kt][p, t] = values_s[t, 128*kt + p]
            vt_tiles = []
            for kt in range(KT):
                vt = vt_pool.tile([128, TS], BF16, name=f"vt{kt}")
                nc.sync.dma_start(vt[:, :], valsT[kt * 128:(kt + 1) * 128, :])
                vt_tiles.append(vt)

            # ---------------- pass 1: matmul + tanh + exp ---------------
            e_tiles = []
            for dj in range(DT):
                e_tiles.append(e_pool.tile([128, TS], FP16, name=f"e{dj}"))

            for dj in range(DT):
                w2tb = w2tb_pool.tile([128, KT * 128], BF16)
                nc.sync.dma_start(w2tb[:, :], w2t[dj, :, :])
                ps_tiles = [psum_pool.tile([128, 512], F32, tag="ps", name=f"ps{i}")
                            for i in range(TC)]
                # k OUTER: stationary operand reused TC times; dj==0 streams
                # at VT-DMA pace.
                for kt in range(KT):
                    for tc_i in range(TC):
                        nc.tensor.matmul(
                            ps_tiles[tc_i][:, :],
                            w2tb[:, kt * 128:(kt + 1) * 128],
                            vt_tiles[kt][:, tc_i * 512:(tc_i + 1) * 512],
                            start=(kt == 0),
                            stop=(kt == KT - 1),
                        )
                for tc_i in range(TC):
                    st = st_pool.tile([128, 512], F32)
                    nc.scalar.activation(
                        st[:, :], ps_tiles[tc_i][:, :],
                        mybir.ActivationFunctionType.Tanh,
                        bias=qbv[:, dj:dj + 1], scale=1.0,
                    )
                    nc.scalar.activation(
                        e_tiles[dj][:, tc_i * 512:(tc_i + 1) * 512], st[:, :],
                        mybir.ActivationFunctionType.Exp,
                        bias=0.0, scale=vv[:, dj:dj + 1],
                        accum_out=acc[:, dj * TC + tc_i:dj * TC + tc_i + 1],
                    )
                nc.vector.tensor_reduce(
                    Sloc[:, dj:dj + 1],
                    acc[:, dj * TC:(dj + 1) * TC],
                    axis=mybir.AxisListType.X,
                    op=mybir.AluOpType.add,
                )

            # ---------------- sum-exp AllReduce + reciprocal ------------
            nc.gpsimd.dma_start(s_in[:, :], Sloc[:, :])
            nc.gpsimd.collective_compute(
                "AllReduce", mybir.AluOpType.add,
                replica_groups=[list(range(N_CORES_))],
                ins=[s_in.opt()], outs=[s_out.opt()],
            )
            nc.gpsimd.dma_start(rv[:, :], s_out[:, :])
            nc.vector.reciprocal(rv[:, :], rv[:, :])

            # ---------------- pass 2 ------------------------------------
            # e *= 1/S (per-partition, in place, fp16 4x mode)
            for dj in range(DT):
                nc.vector.tensor_scalar(
                    out=e_tiles[dj][:, :], in0=e_tiles[dj][:, :],
                    scalar1=rv[:, dj:dj + 1], scalar2=None,
                    op0=mybir.AluOpType.mult)

            # outT = e * valuesT (fp32), transpose back, evacuate, store
            for g in range(NG):
                for th in range(NTH):
                    oT = []
                    for jj in range(GJ):
                        dj = g * GJ + jj
                        vtn = vtn_pool.tile([128, THW], F32)
                        nc.sync.dma_start(
                            vtn[:, :],
                            valsTf[dj * 128:(dj + 1) * 128, th * THW:(th + 1) * THW])
                        ot = outT_pool.tile([128, THW], F32, tag="oT")
                        nc.vector.tensor_mul(
                            ot[:, :],
                            e_tiles[dj][:, th * THW:(th + 1) * THW], vtn[:, :])
                        oT.append(ot)
                    for itl in range(THW // 128):
                        it = th * (THW // 128) + itl
                        pst = psumT_pool.tile([128, GJ * 128], F32, tag="pT")
                        for jj in range(GJ):
                            nc.tensor.transpose(
                                pst[:, jj * 128:(jj + 1) * 128],
                                oT[jj][:, itl * 128:(itl + 1) * 128],
                                ident[:, :],
                            )
                        osb = osb_pool.tile([128, GJ * 128], F32)
                        nc.scalar.copy(osb[:, :], pst[:, :])
                        nc.sync.dma_start(
                            out[it * 128:(it + 1) * 128,
                                g * GJ * 128:(g + 1) * GJ * 128],
                            osb[:, :])

    nc.compile()
    return nc


_NC_CACHE = None


def _get_nc():
    global _NC_CACHE
    if _NC_CACHE is None:
        _NC_CACHE = build_kernel()
    return _NC_CACHE


def make_in_maps(query, values, v, W1_w, W1_b, W2_w, W2_b,
                 D_=None, TS_=None, KS_=None, n_cores=N_CORES):
    import ml_dtypes
    D_ = D_ or D
    TS_ = TS_ or TS
    KS_ = KS_ or KS
    DT_ = D_ // 128
    KT_ = D_ // 128
    # w2t blocked: B[dj, p, kt, f] = W2_w[128dj+f, 128kt+p]
    w2t_blocked = np.ascontiguousarray(
        W2_w.reshape(DT_, 128, KT_, 128).transpose(0, 3, 2, 1)
        .reshape(DT_, 128, KT_ * 128).astype(ml_dtypes.bfloat16))
    in_maps = []
    for c in range(n_cores):
        vs = np.ascontiguousarray(values[c * TS_:(c + 1) * TS_])
        vsT = np.ascontiguousarray(vs.T)
        w1s = W1_w[:, c * KS_:(c + 1) * KS_]
        w1blocked = np.ascontiguousarray(
            w1s.reshape(DT_, 128, KS_).transpose(1, 0, 2).reshape(128, DT_ * KS_))
        in_maps.append({
            "valsTf": vsT,
            "valsT": np.ascontiguousarray(vsT.astype(ml_dtypes.bfloat16)),
            "w2t": w2t_blocked,
            "w1c": w1blocked,
            "qs": np.ascontiguousarray(query[c * KS_:(c + 1) * KS_]),
            "w1b": W1_b,
            "w2b": W2_b,
            "vvec": v,
        })
    return in_maps


def kernel(query, values, v, W1_w, W1_b, W2_w, W2_b, _trace=False, _trace_kwargs=None):
    query = np.asarray(query, np.float32)
    values = np.asarray(values, np.float32)
    v = np.asarray(v, np.float32)
    W1_w = np.asarray(W1_w, np.float32)
    W1_b = np.asarray(W1_b, np.float32)
    W2_w = np.asarray(W2_w, np.float32)
    W2_b = np.asarray(W2_b, np.float32)

    nc = _get_nc()
    in_maps = make_in_maps(query, values, v, W1_w, W1_b, W2_w, W2_b)
    res = run_bass_kernel_spmd(
        nc, in_maps, core_ids=list(range(N_CORES)),
        trace=_trace, **(_trace_kwargs or {}),
    )
    shards = [np.asarray(om["out"], np.float32) for om in res.results]
    out = np.concatenate(shards, axis=0)
    if _trace:
        return out, res
    return out


# revision 18
# speedup vs baseline: 1.2232x; 1.1735x over previous
"""Bahdanau additive-attention kernel for Trainium2, SPMD across 8 NeuronCores.

Reference computation (all fp32):
    q_proj  = query @ W1_w.T + W1_b            # [D]
    v_proj  = values @ W2_w.T + W2_b           # [T, D]
    weights = softmax(tanh(q_proj + v_proj) * v, axis=0)   # over T
    out     = weights * values                 # [T, D]

Sharding: values is split along T across 8 cores (2048 rows each); W2 is
replicated (shipped pre-transposed + pre-blocked in bf16); the q-projection
matvec is sharded over the contraction dim (each core handles 256 columns of
W1) and finished with an AllReduce; the softmax denominator (per-column sum
of exps) is AllReduced.  Logits are bounded in [-0.1, 0.1] (tanh * v with
|v| <= 0.1) so the softmax needs no max-subtraction pass.

Per-core device program:
  - VT (values shard transposed, bf16) resident in SBUF as the moving matmul
    operand; psum tiles are [d=128 part, t=512 free]; the k loop is OUTER so
    the first tiles stream at DMA pace and the stationary operand is reused
    across 4 consecutive matmuls.
  - ScalarE: tanh(psum + qb[d]) then exp(v[d] * x) with accum_out giving the
    per-partition running sum of exps (softmax denominator) for free.
  - e stored fp16 in SBUF.  Pass 2: e *= 1/S[d] (per-partition tensor_scalar,
    in place), outT = e * valuesT(fp32) on DVE, TensorE transposes outT back
    to [t, d], ScalarE evacuates PSUM to SBUF, DMA out.
"""

import numpy as np

import concourse.bacc as bacc
import concourse.bass as bass
import concourse.tile as tile
from concourse import mybir
from concourse import masks
from concourse.bass_utils import run_bass_kernel_spmd

F32 = mybir.dt.float32
BF16 = mybir.dt.bfloat16
FP16 = mybir.dt.float16

D = 2048          # feature dim
T = 16384         # total timesteps
N_CORES = 8
TS = T // N_CORES  # timesteps per core = 2048
KS = D // N_CORES  # W1 contraction slice per core = 256


def build_kernel(D=D, TS=TS, KS=KS, n_cores=N_CORES, debug=False):
    DT = D // 128     # d-tiles of 128
    KT = D // 128     # k-tiles of 128
    TC = TS // 512    # t-chunks of 512
    IT = TS // 128    # t-tiles of 128
    GJ = min(4, DT)   # dj per pass-2 group (one 512-wide d-chunk)
    NG = DT // GJ     # number of pass-2 groups
    THW = min(1024, TS)  # pass-2 t-half width
    NTH = TS // THW
    N_CORES_ = n_cores

    nc = bacc.Bacc(None, target_bir_lowering=False, debug=debug, num_devices=N_CORES_)

    # Per-core inputs (see make_in_maps for host-side layouts)
    valsT = nc.dram_tensor("valsT", [D, TS], FP16, kind="ExternalInput")
    w2t = nc.dram_tensor("w2t", [DT, 128, KT * 128], FP16, kind="ExternalInput")
    w1c = nc.dram_tensor("w1c", [128, DT * KS], F32, kind="ExternalInput")
    qs = nc.dram_tensor("qs", [KS], F32, kind="ExternalInput")
    w1b = nc.dram_tensor("w1b", [D], F32, kind="ExternalInput")
    w2b = nc.dram_tensor("w2b", [D], F32, kind="ExternalInput")
    vvec = nc.dram_tensor("vvec", [D], F32, kind="ExternalInput")
    out = nc.dram_tensor("out", [TS, D], F32, kind="ExternalOutput")

    with tile.TileContext(nc) as tc:
        with (
            tc.tile_pool(name="const", bufs=1) as const_pool,
            tc.tile_pool(name="vt", bufs=1) as vt_pool,
            tc.tile_pool(name="e", bufs=1) as e_pool,
            tc.tile_pool(name="w2tb", bufs=2) as w2tb_pool,
            tc.tile_pool(name="st", bufs=2) as st_pool,
            tc.tile_pool(name="outT", bufs=6) as outT_pool,
            tc.tile_pool(name="osb", bufs=10) as osb_pool,
            tc.tile_pool(name="psum", bufs=6, space="PSUM") as psum_pool,
            tc.tile_pool(name="psumT", bufs=2, space="PSUM") as psumT_pool,
            tc.tile_pool(name="dram", bufs=1, space="DRAM") as dram_pool,
        ):
            # ---------------- constants / small vectors ----------------
            qbv = const_pool.tile([128, DT], F32)    # qb[d] laid out [p, dj]
            vv = const_pool.tile([128, DT], F32)     # v[d]
            rv = const_pool.tile([128, DT], F32)     # 1/S[d]
            Sloc = const_pool.tile([128, DT], F32)   # local sum-exp
            b1v = const_pool.tile([128, DT], F32)
            b2v = const_pool.tile([128, DT], F32)
            acc = const_pool.tile([128, DT * TC], F32)  # per (dj, tc) exp-sums
            ident = const_pool.tile([128, 128], F32)
            ones1 = const_pool.tile([1, 128], F32)
            qs1 = const_pool.tile([1, KS], F32)
            qpart = const_pool.tile([128, DT], F32)  # local q_proj partial

            masks.make_identity(nc, ident[:, :])
            nc.vector.memset(ones1[:, :], 1.0)

            # dram bounce buffers for the two collectives
            qb_in = dram_pool.tile([128, DT], F32)
            qb_out = dram_pool.tile([128, DT], F32)
            s_in = dram_pool.tile([128, DT], F32)
            s_out = dram_pool.tile([128, DT], F32)

            # ---------------- q-projection matvec (sharded over k) -----
            # Feeder DMAs go through gpsimd (SWDGE) so the AllReduce is not
            # queued behind the bulk HWDGE loads.
            W1CH = min(4, DT)
            JCH = DT // W1CH
            with tc.tile_pool(name="w1pool", bufs=2) as w1_pool:
                qsb = w1_pool.tile([128, KS], F32, bufs=1)
                junk = w1_pool.tile([128, KS], F32, bufs=1)
                nc.gpsimd.dma_start(qs1[:, :], qs[:].rearrange("(a k) -> a k", a=1))
                psq = psumT_pool.tile([128, KS], F32, tag="pT")
                nc.tensor.matmul(psq[:, :], ones1[:, :], qs1[:, :], start=True, stop=True)
                nc.scalar.copy(qsb[:, :], psq[:, :])
                for ch in range(W1CH):
                    w1t = w1_pool.tile([128, JCH * KS], F32, tag="w1t")
                    nc.gpsimd.dma_start(
                        w1t[:, :], w1c[:, ch * JCH * KS:(ch + 1) * JCH * KS])
                    for jj in range(JCH):
                        j = ch * JCH + jj
                        nc.vector.tensor_mul(
                            junk[:, :], w1t[:, jj * KS:(jj + 1) * KS], qsb[:, :])
                        nc.vector.tensor_reduce(
                            qpart[:, j:j + 1], junk[:, :],
                            axis=mybir.AxisListType.X, op=mybir.AluOpType.add)

                nc.gpsimd.dma_start(qb_in[:, :], qpart[:, :])
                nc.gpsimd.collective_compute(
                    "AllReduce", mybir.AluOpType.add,
                    replica_groups=[list(range(N_CORES_))],
                    ins=[qb_in.opt()], outs=[qb_out.opt()],
                )

            # biases / v in [p, dj] layout: elem (p, j) <- dram[128j + p]
            nc.gpsimd.dma_start(b1v[:, :], w1b[:].rearrange("(j p) -> p j", p=128))
            nc.gpsimd.dma_start(b2v[:, :], w2b[:].rearrange("(j p) -> p j", p=128))
            nc.gpsimd.dma_start(vv[:, :], vvec[:].rearrange("(j p) -> p j", p=128))
            nc.vector.tensor_add(b1v[:, :], b1v[:, :], b2v[:, :])  # off critical path
            nc.gpsimd.dma_start(qbv[:, :], qb_out[:, :])
            nc.vector.tensor_add(qbv[:, :], qbv[:, :], b1v[:, :])

            # ---------------- VT resident load (bf16) -------------------
            # vt[kt][p, t] = values_s[t, 128*kt + p]
            vt_tiles = []
            for kt in range(KT):
                vt = vt_pool.tile([128, TS], FP16, name=f"vt{kt}")
                for c4 in range(TC):
                    nc.sync.dma_start(
                        vt[:, c4 * 512:(c4 + 1) * 512],
                        valsT[kt * 128:(kt + 1) * 128, c4 * 512:(c4 + 1) * 512])
                vt_tiles.append(vt)

            # ---------------- pass 1: matmul + tanh + exp ---------------
            e_tiles = []
            for dj in range(DT):
                e_tiles.append(e_pool.tile([128, TS], FP16, name=f"e{dj}"))

            for dj in range(DT):
                w2tb = w2tb_pool.tile([128, KT * 128], FP16)
                nc.sync.dma_start(w2tb[:, :], w2t[dj, :, :])
                ps_tiles = [psum_pool.tile([128, 512], F32, tag="ps", name=f"ps{i}")
                            for i in range(TC)]
                # k OUTER: stationary operand reused TC times; dj==0 streams
                # at VT-DMA pace.
                for kt in range(KT):
                    for tc_i in range(TC):
                        nc.tensor.matmul(
                            ps_tiles[tc_i][:, :],
                            w2tb[:, kt * 128:(kt + 1) * 128],
                            vt_tiles[kt][:, tc_i * 512:(tc_i + 1) * 512],
                            start=(kt == 0),
                            stop=(kt == KT - 1),
                        )
                for tc_i in range(TC):
                    st = st_pool.tile([128, 512], F32)
                    nc.scalar.activation(
                        st[:, :], ps_tiles[tc_i][:, :],
                        mybir.ActivationFunctionType.Tanh,
                        bias=qbv[:, dj:dj + 1], scale=1.0,
                    )
                    nc.scalar.activation(
                        e_tiles[dj][:, tc_i * 512:(tc_i + 1) * 512], st[:, :],
                        mybir.ActivationFunctionType.Exp,
                        bias=0.0, scale=vv[:, dj:dj + 1],
                        accum_out=acc[:, dj * TC + tc_i:dj * TC + tc_i + 1],
                    )
                nc.vector.tensor_reduce(
                    Sloc[:, dj:dj + 1],
                    acc[:, dj * TC:(dj + 1) * TC],
                    axis=mybir.AxisListType.X,
                    op=mybir.AluOpType.add,
                )

            # ---------------- sum-exp AllReduce + reciprocal ------------
            nc.gpsimd.dma_start(s_in[:, :], Sloc[:, :])
            nc.gpsimd.collective_compute(
                "AllReduce", mybir.AluOpType.add,
                replica_groups=[list(range(N_CORES_))],
                ins=[s_in.opt()], outs=[s_out.opt()],
            )
            nc.gpsimd.dma_start(rv[:, :], s_out[:, :])
            nc.vector.reciprocal(rv[:, :], rv[:, :])

            # ---------------- pass 2 ------------------------------------
            # e *= 1/S (per-partition, in place, fp16 4x mode), then
            # outT = e * valuesT (resident fp16 VT), transpose back, store.
            ndma = 0
            for g in range(NG):
                for th in range(NTH):
                    oT = []
                    for jj in range(GJ):
                        dj = g * GJ + jj
                        if th == 0:
                            nc.vector.tensor_scalar(
                                out=e_tiles[dj][:, :], in0=e_tiles[dj][:, :],
                                scalar1=rv[:, dj:dj + 1], scalar2=None,
                                op0=mybir.AluOpType.mult)
                        ot = outT_pool.tile([128, THW], F32, tag="oT")
                        nc.vector.tensor_mul(
                            ot[:, :],
                            e_tiles[dj][:, th * THW:(th + 1) * THW],
                            vt_tiles[dj][:, th * THW:(th + 1) * THW])
                        oT.append(ot)
                    for itl in range(THW // 128):
                        it = th * (THW // 128) + itl
                        pst = psumT_pool.tile([128, GJ * 128], F32, tag="pT")
                        for jj in range(GJ):
                            nc.tensor.transpose(
                                pst[:, jj * 128:(jj + 1) * 128],
                                oT[jj][:, itl * 128:(itl + 1) * 128],
                                ident[:, :],
                            )
                        osb = osb_pool.tile([128, GJ * 128], F32)
                        nc.scalar.copy(osb[:, :], pst[:, :])
                        eng = nc.sync if ndma % 2 == 0 else nc.gpsimd
                        ndma += 1
                        eng.dma_start(
                            out[it * 128:(it + 1) * 128,
                                g * GJ * 128:(g + 1) * GJ * 128],
                            osb[:, :])

    nc.compile()
    return nc


_NC_CACHE = None


def _get_nc():
    global _NC_CACHE
    if _NC_CACHE is None:
        _NC_CACHE = build_kernel()
    return _NC_CACHE


def make_in_maps(query, values, v, W1_w, W1_b, W2_w, W2_b,
                 D_=None, TS_=None, KS_=None, n_cores=N_CORES):
    import ml_dtypes
    D_ = D_ or D
    TS_ = TS_ or TS
    KS_ = KS_ or KS
    DT_ = D_ // 128
    KT_ = D_ // 128
    # w2t blocked: B[dj, p, kt, f] = W2_w[128dj+f, 128kt+p]
    w2t_blocked = np.ascontiguousarray(
        W2_w.reshape(DT_, 128, KT_, 128).transpose(0, 3, 2, 1)
        .reshape(DT_, 128, KT_ * 128).astype(np.float16))
    in_maps = []
    for c in range(n_cores):
        vs = np.ascontiguousarray(values[c * TS_:(c + 1) * TS_])
        vsT = np.ascontiguousarray(vs.T.astype(np.float16))
        w1s = W1_w[:, c * KS_:(c + 1) * KS_]
        w1blocked = np.ascontiguousarray(
            w1s.reshape(DT_, 128, KS_).transpose(1, 0, 2).reshape(128, DT_ * KS_))
        in_maps.append({
            "valsT": vsT,
            "w2t": w2t_blocked,
            "w1c": w1blocked,
            "qs": np.ascontiguousarray(query[c * KS_:(c + 1) * KS_]),
            "w1b": W1_b,
            "w2b": W2_b,
            "vvec": v,
        })
    return in_maps


def kernel(query, values, v, W1_w, W1_b, W2_w, W2_b, _trace=False, _trace_kwargs=None):
    query = np.asarray(query, np.float32)
    values = np.asarray(values, np.float32)
    v = np.asarray(v, np.float32)
    W1_w = np.asarray(W1_w, np.float32)
    W1_b = np.asarray(W1_b, np.float32)
    W2_w = np.asarray(W2_w, np.float32)
    W2_b = np.asarray(W2_b, np.float32)

    nc = _get_nc()
    in_maps = make_in_maps(query, values, v, W1_w, W1_b, W2_w, W2_b)
    res = run_bass_kernel_spmd(
        nc, in_maps, core_ids=list(range(N_CORES)),
        trace=_trace, **(_trace_kwargs or {}),
    )
    shards = [np.asarray(om["out"], np.float32) for om in res.results]
    out = np.concatenate(shards, axis=0)
    if _trace:
        return out, res
    return out
